# revision 1
# baseline (speedup 1.0000x reference)
"""NSA (native sparse attention) forward kernel for Trainium2, 8 NeuronCores.

Sharding: tensor-parallel over query heads; core c owns heads {2c, 2c+1}.
The shared per-token top-k block selection (summed over all 16 heads) is
recomputed on every core; sel/swa/cmp attention outputs only for own heads.

v2 design vs baseline:
- fp16 operands for all matmuls (1 cyc/row on PE vs 4 for fp32), fp32 PSUM
  accumulation. End-to-end rel err ~6e-3 (gate 2e-2).
- Scores computed transposed [key, query], both own heads batched 256-wide.
- Selection masking folded INTO the QK scores: a second matmul accumulates
  E2c @ ((keep-1)*60) into the score PSUM, so exp() directly produces
  masked probabilities (masked entries underflow fp16 to 0). No per-chunk
  mask multiply, no DRAM round-trip.
- swa shares the chunk loop; window chunks add one raw-score matmul.
- Software pipelining: cmp/top-k stage of tile qt runs interleaved with the
  sel/swa stage of tile qt-1, hiding the top-k serial tail from the PE.
- PSUM: one [128, 2, 512] ring (2 bufs, 4 banks) shared by all score/slc
  work + 4 dedicated accumulator banks for sel/swa PV chains.
- GPSIMD(Pool) touches SBUF only (HW cannot access PSUM from Pool).
- All sigmoids hoisted to start (2 activation table loads total).
"""

import numpy as np

import concourse.bacc as bacc
import concourse.bass as bass
import concourse.tile as tile
from concourse import mybir
from concourse.bass_utils import run_bass_kernel_spmd

# ---- problem constants (hardcoded per contest rules) ----
T, HQ, D = 2048, 16, 128
KS, ST, BS = 32, 16, 64
TOP_N, NINIT, NLOCAL, WIN = 16, 2, 1, 512
M = (T - KS) // ST + 1          # 127 compressed tokens
NB = T // BS                    # 32 selection blocks
NT = T // 128                   # 16 query tiles
NCORES = 8
HPC = HQ // NCORES              # 2 heads per core
SCALE = float(D) ** -0.5
NEGC = -1e30
BIGC = 1e30
EPS = 1e-30
ZAP = -1e38
MASKNEG = 60.0                  # (keep-1)*60 added to scores pre-exp
F32 = mybir.dt.float32
F16 = mybir.dt.float16

WCH = NT // (WIN // 128)        # swa window spans 4 full chunks + diag


def _build_program():
    nc = bacc.Bacc("TRN2", target_bir_lowering=False, debug=False,
                   num_devices=NCORES)

    d = {}
    d["qt_all"] = nc.dram_tensor("qt_all", [NT, 128, HQ, 128], F16, kind="ExternalInput").ap()
    d["q_own"] = nc.dram_tensor("q_own", [NT, 128, HPC, 128], F16, kind="ExternalInput").ap()
    d["kT"] = nc.dram_tensor("kT", [128, T], F16, kind="ExternalInput").ap()
    d["v_ext"] = nc.dram_tensor("v_ext", [128, NT, 129], F16, kind="ExternalInput").ap()
    d["cmp_kT"] = nc.dram_tensor("cmp_kT", [128, M], F16, kind="ExternalInput").ap()
    d["cmp_rhs"] = nc.dram_tensor("cmp_rhs", [M, 161], F16, kind="ExternalInput").ap()
    d["cmp_keepT"] = nc.dram_tensor("cmp_keepT", [M, NT, 128], F16, kind="ExternalInput").ap()
    d["slc_keep"] = nc.dram_tensor("slc_keep", [128, NT, NB], F32, kind="ExternalInput").ap()
    d["slc_ovr"] = nc.dram_tensor("slc_ovr", [128, NT, NB], F32, kind="ExternalInput").ap()
    d["tri_i2"] = nc.dram_tensor("tri_i2", [128, HPC * 128], F16, kind="ExternalInput").ap()
    d["tri_s2"] = nc.dram_tensor("tri_s2", [128, HPC * 128], F16, kind="ExternalInput").ap()
    d["e2c"] = nc.dram_tensor("e2c", [NB, NT, 128], F16, kind="ExternalInput").ap()
    d["ident"] = nc.dram_tensor("ident", [128, 128], F32, kind="ExternalInput").ap()
    d["cw_own"] = nc.dram_tensor("cw_own", [128, NT, 3 * HPC], F32, kind="ExternalInput").ap()
    out_ap = nc.dram_tensor("out", [T, HPC, 128], F32, kind="ExternalOutput").ap()

    from contextlib import ExitStack
    with tile.TileContext(nc) as tc:
        with ExitStack() as ctx:
            _body(tc, nc, d, out_ap, ctx)
    nc.compile()
    return nc


def _body(tc, nc, d, out_ap, ctx):
    EXP = mybir.ActivationFunctionType.Exp
    SIG = mybir.ActivationFunctionType.Sigmoid
    MUL = mybir.AluOpType.mult
    ADD = mybir.AluOpType.add
    ISEQ = mybir.AluOpType.is_equal

    const = ctx.enter_context(tc.tile_pool(name="const", bufs=1))
    qtp = ctx.enter_context(tc.tile_pool(name="qtp", bufs=2))
    qop = ctx.enter_context(tc.tile_pool(name="qop", bufs=3))
    probs = ctx.enter_context(tc.tile_pool(name="probs", bufs=3))
    sprobs = ctx.enter_context(tc.tile_pool(name="sprobs", bufs=3))
    slabp = ctx.enter_context(tc.tile_pool(name="slabp", bufs=2))
    keepp = ctx.enter_context(tc.tile_pool(name="keepp", bufs=3))
    smallp = ctx.enter_context(tc.tile_pool(name="smallp", bufs=2))
    outp = ctx.enter_context(tc.tile_pool(name="outp", bufs=3))
    # PSUM: qk ring [128,2,512] x2 bufs = 4 banks + sel0/sel1/swa0/swa1 = 8
    pp_qk = ctx.enter_context(tc.tile_pool(name="pp_qk", bufs=2, space="PSUM"))
    pp_acc = ctx.enter_context(tc.tile_pool(name="pp_acc", bufs=1, space="PSUM"))

    # ---- load constants ----
    kT_sb = const.tile([128, T], F16, tag="kT")
    nc.sync.dma_start(out=kT_sb, in_=d["kT"])
    vext_sb = const.tile([128, NT, 129], F16, tag="vext")
    nc.sync.dma_start(out=vext_sb, in_=d["v_ext"])
    cmpkT_sb = const.tile([128, M], F16, tag="cmpkT")
    nc.sync.dma_start(out=cmpkT_sb, in_=d["cmp_kT"])
    cmprhs_sb = const.tile([M, 161], F16, tag="cmprhs")
    nc.sync.dma_start(out=cmprhs_sb, in_=d["cmp_rhs"])
    cmpkeepT_sb = const.tile([M, NT, 128], F16, tag="cmpkeepT")
    nc.sync.dma_start(out=cmpkeepT_sb, in_=d["cmp_keepT"])
    slckeep_sb = const.tile([128, NT, NB], F32, tag="slckeep")
    nc.sync.dma_start(out=slckeep_sb, in_=d["slc_keep"])
    slcovr_sb = const.tile([128, NT, NB], F32, tag="slcovr")
    nc.sync.dma_start(out=slcovr_sb, in_=d["slc_ovr"])
    tri_i2_sb = const.tile([128, HPC * 128], F16, tag="tri_i2")
    nc.sync.dma_start(out=tri_i2_sb, in_=d["tri_i2"])
    tri_s2_sb = const.tile([128, HPC * 128], F16, tag="tri_s2")
    nc.sync.dma_start(out=tri_s2_sb, in_=d["tri_s2"])
    e2c_sb = const.tile([NB, NT, 128], F16, tag="e2c")
    nc.sync.dma_start(out=e2c_sb, in_=d["e2c"])
    ident_sb = const.tile([128, 128], F32, tag="ident")
    nc.sync.dma_start(out=ident_sb, in_=d["ident"])
    cw_sb = const.tile([128, NT, 3 * HPC], F32, tag="cw")
    nc.sync.dma_start(out=cw_sb, in_=d["cw_own"])
    sg_sb = const.tile([128, NT, 3 * HPC], F32, tag="sg")
    nc.scalar.activation(sg_sb, cw_sb, SIG)

    def phase_A(qt):
        """cmp scores + slc accumulation + top-k for tile qt.

        Returns tiles needed by phase_B(qt): keep01, qo_sb, pTo."""
        qt_sb = qtp.tile([128, HQ, 128], F16, tag="qt")
        nc.sync.dma_start(out=qt_sb, in_=d["qt_all"][qt])
        qo_sb = qop.tile([128, HPC, 128], F16, tag="qo")
        nc.sync.dma_start(out=qo_sb, in_=d["q_own"][qt])

        keep_bc = (cmpkeepT_sb[:, qt, :].unsqueeze(1).unsqueeze(1)
                   .to_broadcast((M, 2, 4, 128)))
        pT4s = []
        for pair in range(2):
            ps = pp_qk.tile([128, 2, 512], F32, tag="qk", name=f"ps_cqk{pair}")
            for half in range(2):
                b = pair * 2 + half
                nc.tensor.matmul(ps[0:M, half, :], lhsT=cmpkT_sb,
                                 rhs=qt_sb[:, 4 * b:4 * b + 4, :],
                                 start=True, stop=True)
            pT4 = probs.tile([128, 2, 4, 128], F16, tag="p4", name=f"pT4_{pair}")
            nc.scalar.activation(
                pT4[0:M], ps[0:M].rearrange("p a (b c) -> p a b c", b=4),
                EXP, scale=SCALE)
            nc.gpsimd.tensor_mul(pT4[0:M], pT4[0:M], keep_bc)
            pT4s.append(pT4)

        # own-head cmp probabilities (for cmp_o later, in phase_B)
        ps = pp_qk.tile([128, 2, 512], F32, tag="qk", name="ps_oqk")
        nc.tensor.matmul(ps[0:M, 0, 0:256], lhsT=cmpkT_sb, rhs=qo_sb,
                         start=True, stop=True)
        pTo = probs.tile([128, HPC, 128], F16, tag="pTo")
        nc.scalar.activation(
            pTo[0:M], ps[0:M, 0, 0:256].rearrange("p (a b) -> p a b", a=2),
            EXP, scale=SCALE)
        nc.gpsimd.tensor_mul(
            pTo[0:M], pTo[0:M],
            cmpkeepT_sb[:, qt, :].unsqueeze(1).to_broadcast((M, 2, 128)))

        # slc numerators + denominators: 16 heads x 33 cols, 8 per bank
        slc_ps = pp_qk.tile([128, 2, 512], F32, tag="qk", name="ps_slc")
        for h in range(16):
            bank, h8 = divmod(h, 8)
            nc.tensor.matmul(slc_ps[:, bank, 33 * h8:33 * h8 + 33],
                             lhsT=pT4s[h // 8][0:M, (h % 8) // 4, h % 4, :],
                             rhs=cmprhs_sb[:, 0:33],
                             start=(h8 == 0), stop=(h8 == 7))
        slab = slabp.tile([128, 2, 8, 33], F32, tag="slab")
        for bank in range(2):
            nc.vector.tensor_copy(
                slab[:, bank],
                slc_ps[:, bank, 0:264].rearrange("p (h w) -> p h w", h=8))

        rc16 = smallp.tile([128, 2, 8], F32, tag="rc16")
        if qt == 0:
            dn16 = smallp.tile([128, 2, 8], F32, tag="dn16")
            nc.vector.tensor_scalar(dn16, slab[:, :, :, 32], EPS, None, op0=ADD)
            nc.vector.reciprocal(rc16, dn16)
        else:
            nc.vector.reciprocal(rc16, slab[:, :, :, 32])

        nc.gpsimd.tensor_mul(slab[:, :, :, 0:32], slab[:, :, :, 0:32],
                             rc16.unsqueeze(3).to_broadcast((128, 2, 8, 32)))
        slc_fin = keepp.tile([128, NB], F32, tag="slc_fin")
        nc.vector.tensor_reduce(slc_fin,
                                slab[:, :, :, 0:32].rearrange(
                                    "p a h w -> p w (a h)"),
                                axis=mybir.AxisListType.X, op=ADD)
        nc.gpsimd.tensor_mul(slc_fin, slc_fin, slckeep_sb[:, qt, :])
        nc.gpsimd.tensor_add(slc_fin, slc_fin, slcovr_sb[:, qt, :])

        # top-k (exactly the proven baseline sequence)
        z1 = keepp.tile([128, NB], F32, tag="z1")
        mx = keepp.tile([128, 8], F32, tag="mx")
        nc.vector.max(mx, slc_fin)
        nc.vector.match_replace(z1, mx, slc_fin, ZAP)
        z2 = keepp.tile([128, NB], F32, tag="z2")
        mx2 = keepp.tile([128, 8], F32, tag="mx2")
        nc.vector.max(mx2, z1)
        nc.vector.match_replace(z2, mx2, z1, ZAP)
        keep01 = keepp.tile([128, NB], F32, tag="keep01")
        nc.vector.tensor_tensor(keep01, slc_fin, z2, op=ISEQ)  # 1 = NOT selected
        nc.vector.tensor_scalar(keep01, keep01, -1.0, 1.0, op0=MUL, op1=ADD)
        return dict(keep01=keep01, qo_sb=qo_sb, pTo=pTo)

    def phase_B(j, st):
        """sel + swa attention and output combine for tile j."""
        keep01, qo_sb, pTo = st["keep01"], st["qo_sb"], st["pTo"]

        # keepT2neg[b, rep, q] = (keep01[q, b] - 1) * 60
        ps_t = pp_qk.tile([128, 2, 512], F32, tag="qk", name="ps_tr")
        nc.tensor.transpose(ps_t[0:NB, 0, 0:128], keep01, ident_sb)
        ktn = keepp.tile([NB, HPC, 128], F16, tag="ktn")
        for rep in range(HPC):
            nc.vector.tensor_scalar(ktn[:, rep, :], ps_t[0:NB, 0, 0:128],
                                    -1.0, MASKNEG, op0=ADD, op1=MUL)

        ps_sel = [pp_acc.tile([128, 129], F32, tag=f"sel{oh}", name=f"ps_sel{oh}")
                  for oh in range(HPC)]
        ps_swa = [pp_acc.tile([128, 129], F32, tag=f"swa{oh}", name=f"ps_swa{oh}")
                  for oh in range(HPC)]
        c_lo = max(0, j - WCH)

        # pack chunks into 4-slot generations (window chunks need 2 slots)
        gens, cur, used = [], [], 0
        for c in range(j + 1):
            need = 2 if c >= c_lo else 1
            if used + need > 4:
                gens.append(cur)
                cur, used = [], 0
            cur.append((c, used))
            used += need
        if cur:
            gens.append(cur)

        def emit_pvs(pvl):
            for c, sel_st, swa_st in pvl:
                w = c >= c_lo
                for oh in range(HPC):
                    nc.tensor.matmul(ps_sel[oh],
                                     lhsT=sel_st[:, 128 * oh:128 * oh + 128],
                                     rhs=vext_sb[:, c, :],
                                     start=(c == 0), stop=(c == j))
                    if w:
                        nc.tensor.matmul(ps_swa[oh],
                                         lhsT=swa_st[:, 128 * oh:128 * oh + 128],
                                         rhs=vext_sb[:, c, :],
                                         start=(c == c_lo), stop=(c == j))

        pend = None
        for gi, gen in enumerate(gens):
            t = pp_qk.tile([128, 2, 512], F32, tag="qk", name=f"ps_g{j}_{gi}")

            def region(slot):
                return t[:, slot // 2, (slot % 2) * 256:(slot % 2) * 256 + 256]

            # qk matmuls first, then mask matmuls; group flags per bank
            mms = []  # (bank, kind, region, c)
            nslots = [0, 0]
            for c, slot in gen:
                mms.append((slot // 2, "qk", region(slot), c))
                nslots[slot // 2] += 1
                if c >= c_lo:
                    rs = slot + 1
                    mms.append((rs // 2, "qk", region(rs), c))
                    nslots[rs // 2] += 1
            for c, slot in gen:
                mms.append((slot // 2, "mask", region(slot), c))
            first = {}
            last = {}
            for i, (bank, _, _, _) in enumerate(mms):
                first.setdefault(bank, i)
                last[bank] = i
            for i, (bank, kind, reg, c) in enumerate(mms):
                start = (i == first[bank])
                stop = (i == last[bank])
                if kind == "qk":
                    nc.tensor.matmul(reg, lhsT=kT_sb[:, 128 * c:128 * c + 128],
                                     rhs=qo_sb, start=start, stop=stop)
                else:
                    nc.tensor.matmul(reg, lhsT=e2c_sb[:, c, :], rhs=ktn,
                                     start=start, stop=stop)

            ex2 = sprobs.tile([128, 2, 512], F16, tag="ex", name=f"ex{j}_{gi}")
            for bank in range(2):
                n = nslots[bank]
                if n:
                    nc.scalar.activation(ex2[:, bank, 0:256 * n],
                                         t[:, bank, 0:256 * n], EXP,
                                         scale=SCALE)

            pvl = []
            for c, slot in gen:
                w = c >= c_lo
                sel_sl = ex2[:, slot // 2, (slot % 2) * 256:(slot % 2) * 256 + 256]
                raw_sl = None
                if w:
                    rs = slot + 1
                    raw_sl = ex2[:, rs // 2, (rs % 2) * 256:(rs % 2) * 256 + 256]
                if c == j:  # diagonal: causal tri on both streams
                    pse = sprobs.tile([128, 256], F16, tag="pse")
                    nc.gpsimd.tensor_mul(pse, sel_sl, tri_i2_sb)
                    pw = sprobs.tile([128, 256], F16, tag="pw")
                    nc.gpsimd.tensor_mul(pw, raw_sl, tri_i2_sb)
                    sel_st, swa_st = pse, pw
                elif w and c == c_lo and j >= WCH:  # window start
                    pw = sprobs.tile([128, 256], F16, tag="pw")
                    nc.gpsimd.tensor_mul(pw, raw_sl, tri_s2_sb)
                    sel_st, swa_st = sel_sl, pw
                elif w:
                    sel_st, swa_st = sel_sl, raw_sl
                else:
                    sel_st, swa_st = sel_sl, None
                pvl.append((c, sel_st, swa_st))

            if pend is not None:
                emit_pvs(pend)
            pend = pvl
        if pend is not None:
            emit_pvs(pend)

        # own-head cmp PV + combine
        cpo = pp_qk.tile([128, 2, 512], F32, tag="qk", name="ps_cpo")
        for oh in range(HPC):
            nc.tensor.matmul(cpo[:, oh, 0:129], lhsT=pTo[0:M, oh, :],
                             rhs=cmprhs_sb[:, 32:161], start=True, stop=True)
        for oh in range(HPC):
            rc0 = smallp.tile([128, 1], F32, tag="rc0")
            if j == 0:
                dn = smallp.tile([128, 1], F32, tag="dn")
                nc.vector.tensor_scalar(dn, cpo[:, oh, 0:1], EPS, None, op0=ADD)
                nc.vector.reciprocal(rc0, dn)
            else:
                nc.vector.reciprocal(rc0, cpo[:, oh, 0:1])
            rc1 = smallp.tile([128, 1], F32, tag="rc1")
            nc.vector.reciprocal(rc1, ps_sel[oh][:, 128:129])
            rc2 = smallp.tile([128, 1], F32, tag="rc2")
            nc.vector.reciprocal(rc2, ps_swa[oh][:, 128:129])
            rs0 = smallp.tile([128, 1], F32, tag="rs0")
            nc.vector.tensor_mul(rs0, rc0, sg_sb[:, j, 3 * oh:3 * oh + 1])
            rs1 = smallp.tile([128, 1], F32, tag="rs1")
            nc.vector.tensor_mul(rs1, rc1, sg_sb[:, j, 3 * oh + 1:3 * oh + 2])
            rs2 = smallp.tile([128, 1], F32, tag="rs2")
            nc.vector.tensor_mul(rs2, rc2, sg_sb[:, j, 3 * oh + 2:3 * oh + 3])
            acc_a = outp.tile([128, 128], F32, tag="acc_a")
            nc.vector.tensor_scalar(acc_a, cpo[:, oh, 1:129], rs0, None, op0=MUL)
            acc_b = outp.tile([128, 128], F32, tag="acc_b")
            nc.vector.scalar_tensor_tensor(acc_b, ps_sel[oh][:, 0:128], rs1,
                                           acc_a, op0=MUL, op1=ADD)
            ot = outp.tile([128, 128], F32, tag="ot")
            nc.vector.scalar_tensor_tensor(ot, ps_swa[oh][:, 0:128], rs2,
                                           acc_b, op0=MUL, op1=ADD)
            nc.sync.dma_start(out=out_ap[j * 128:(j + 1) * 128, oh, :], in_=ot)

    prev = None
    for qt in range(NT):
        st = phase_A(qt)
        if prev is not None:
            phase_B(qt - 1, prev)
        prev = st
    phase_B(NT - 1, prev)


def _host_inputs(q, k, v, cw):
    """Precompute per-core input arrays (numpy only; no FLOP-heavy work)."""
    q = np.ascontiguousarray(q, np.float32)
    k2 = np.ascontiguousarray(k[:, 0, :], np.float32)
    v2 = np.ascontiguousarray(v[:, 0, :], np.float32)
    cw = np.ascontiguousarray(cw, np.float32)
    f16 = np.float16

    qt_all = np.ascontiguousarray(
        q.reshape(NT, 128, HQ, D).transpose(0, 3, 2, 1)).astype(f16)
    kT = np.ascontiguousarray(k2.T).astype(f16)
    v_ext = np.ascontiguousarray(
        np.concatenate([v2, np.ones((T, 1), np.float32)], 1)
        .reshape(NT, 128, 129).transpose(1, 0, 2)).astype(f16)
    idx = np.arange(M)[:, None] * ST + np.arange(KS)[None, :]
    cmp_k = k2[idx].mean(1)
    cmp_v = v2[idx].mean(1)
    cmp_kT = np.ascontiguousarray(cmp_k.T).astype(f16)
    ju, bu = KS // ST, BS // ST
    j = np.arange(M)[:, None]
    b = np.arange(NB)[None, :]
    ov = np.maximum(0, np.minimum(j + ju, (b + 1) * bu) - np.maximum(j, b * bu))
    Wmap = (ov / ju).astype(np.float32)
    cmp_rhs = np.ascontiguousarray(
        np.concatenate([Wmap, np.ones((M, 1), np.float32), cmp_v], 1)).astype(f16)
    t_pos = np.arange(T)
    cmp_keepT = np.ascontiguousarray(
        ((np.arange(M)[:, None] * ST + KS) <= (t_pos[None, :] + 1))
        .astype(np.float32).reshape(M, NT, 128)).astype(f16)
    cur_blk = t_pos // BS
    bidx = np.arange(NB)
    future = bidx[None, :] > cur_blk[:, None]
    forced = (bidx[None, :] < NINIT) | (
        (bidx[None, :] <= cur_blk[:, None])
        & (bidx[None, :] > cur_blk[:, None] - NLOCAL))
    slc_keep = np.ascontiguousarray(
        (~(future | forced)).astype(np.float32)
        .reshape(NT, 128, NB).transpose(1, 0, 2))
    slc_ovr = np.ascontiguousarray(
        np.where(forced, BIGC, np.where(future, NEGC, 0.0)).astype(np.float32)
        .reshape(NT, 128, NB).transpose(1, 0, 2))
    ar = np.arange(128)
    tri_i = (ar[None, :] >= ar[:, None]).astype(np.float32)
    tri_s = (ar[None, :] < ar[:, None]).astype(np.float32)
    tri_i2 = np.ascontiguousarray(
        np.broadcast_to(tri_i[:, None, :], (128, HPC, 128))
        .reshape(128, HPC * 128)).astype(f16)
    tri_s2 = np.ascontiguousarray(
        np.broadcast_to(tri_s[:, None, :], (128, HPC, 128))
        .reshape(128, HPC * 128)).astype(f16)
    c_idx = np.arange(NT)
    e2c = np.ascontiguousarray(
        (bidx[:, None, None] ==
         (2 * c_idx[None, :, None] + ar[None, None, :] // 64)).astype(f16))
    ident = np.eye(128, dtype=np.float32)

    shared = dict(qt_all=qt_all, kT=kT, v_ext=v_ext, cmp_kT=cmp_kT,
                  cmp_rhs=cmp_rhs, cmp_keepT=cmp_keepT, slc_keep=slc_keep,
                  slc_ovr=slc_ovr, tri_i2=tri_i2, tri_s2=tri_s2, e2c=e2c,
                  ident=ident)
    in_maps = []
    for c in range(NCORES):
        hs = slice(c * HPC, (c + 1) * HPC)
        q_own = np.ascontiguousarray(
            q[:, hs, :].reshape(NT, 128, HPC, D).transpose(0, 3, 2, 1)).astype(f16)
        cw_own = np.ascontiguousarray(
            cw[:, hs, :].reshape(NT, 128, 3 * HPC).transpose(1, 0, 2))
        in_maps.append(dict(shared, q_own=q_own, cw_own=cw_own))
    return in_maps


_PROGRAM = None


def _get_program():
    global _PROGRAM
    if _PROGRAM is None:
        _PROGRAM = _build_program()
    return _PROGRAM


def kernel(q, k, v, combine_weight, cu_seqlens, _trace=False):
    nc = _get_program()
    in_maps = _host_inputs(np.asarray(q), np.asarray(k), np.asarray(v),
                           np.asarray(combine_weight))
    res = run_bass_kernel_spmd(nc, in_maps, core_ids=list(range(NCORES)),
                               trace=_trace)
    outs = [res.results[c]["out"] for c in range(NCORES)]
    full = np.concatenate(outs, axis=1).astype(np.float32)
    if _trace:
        kernel._last_results = res
    return full



# revision 21
# speedup vs baseline: 1.7606x; 1.7606x over previous
"""NSA (native sparse attention) forward kernel for Trainium2, 8 NeuronCores.

v3 design (vs v2 baseline at 288us):
- Head-TP for sel/swa attention (core c owns heads {2c, 2c+1}) -- keeps the
  program SPMD-uniform.
- The cmp/top-k pipeline (which needs all 16 heads) is SHARDED BY QUERY TILE
  (core c computes top-k for tiles {2c, 2c+1} only, 8x less work than the v2
  all-tiles-on-every-core scheme) and the tiny keep masks (16KB/core) are
  AllGather'd across the 8 cores. The collective latency hides under the
  top-k-independent swa + cmp-own-head work.
- Window chunks share ONE set of raw scores between sel and swa: exp(raw)
  probs are kept in SBUF; sel probs = probs * keep-mask (gpsimd multiply with
  an exact {0,1} mask built by a tiny PE matmul), swa start-chunk probs =
  probs * anti-tri constant. This kills the separate swa score matmuls and
  all per-chunk mask matmuls on window chunks.
- Causal diag mask and cmp validity masks folded into the PE score
  accumulation (constant -MASKNEG matmuls) instead of gpsimd multiplies.
- Activations batched 4 chunks wide ([128,1024]+) to amortize the 352-cycle
  ACT fixed overhead (v2 paid it per 256-wide slot).
- PSUM: ring of 2-bank score tiles (bufs=3) + 1-bank PV accumulators
  (bufs=2) = 8 banks exactly.
"""

import numpy as np

import concourse.bacc as bacc
import concourse.bass as bass
import concourse.tile as tile
from concourse import mybir
from concourse.bass_utils import run_bass_kernel_spmd

# ---- problem constants (hardcoded per contest rules) ----
T, HQ, D = 2048, 16, 128
KS, ST, BS = 32, 16, 64
TOP_N, NINIT, NLOCAL, WIN = 16, 2, 1, 512
M = (T - KS) // ST + 1          # 127 compressed tokens
NB = T // BS                    # 32 selection blocks
NT = T // 128                   # 16 query tiles
NCORES = 8
HPC = HQ // NCORES              # 2 heads per core
SCALE = float(D) ** -0.5
NEGC = -1e30
BIGC = 1e30
EPS = 1e-30
ZAP = -1e38
MASKNEG = 400.0                 # -(1-keep)*400: exp underflows fp16 to exact 0
F32 = mybir.dt.float32
F16 = mybir.dt.float16

WCH = WIN // 128                # window spans 4 full chunks + diag


import os
DEBUG = bool(os.environ.get("KDBG"))


def _build_program():
    nc = bacc.Bacc("TRN2", target_bir_lowering=False, debug=False,
                   num_devices=NCORES)

    d = {}
    # shared constants
    d["kT"] = nc.dram_tensor("kT", [128, T], F16, kind="ExternalInput").ap()
    d["v_ext"] = nc.dram_tensor("v_ext", [128, NT, 129], F16, kind="ExternalInput").ap()
    d["cmp_kT"] = nc.dram_tensor("cmp_kT", [128, M], F16, kind="ExternalInput").ap()
    d["cmp_rhs"] = nc.dram_tensor("cmp_rhs", [M, 162], F16, kind="ExternalInput").ap()
    d["e2c"] = nc.dram_tensor("e2c", [NB, NT, 128], F16, kind="ExternalInput").ap()
    d["ident"] = nc.dram_tensor("ident", [128, 128], F32, kind="ExternalInput").ap()
    d["repI2"] = nc.dram_tensor("repI2", [128, 256], F16, kind="ExternalInput").ap()
    d["repI16"] = nc.dram_tensor("repI16", [128, 2048], F16, kind="ExternalInput").ap()
    d["triNegT"] = nc.dram_tensor("triNegT", [128, 128], F16, kind="ExternalInput").ap()
    d["triS01"] = nc.dram_tensor("triS01", [128, 256], F16, kind="ExternalInput").ap()
    d["cmp_maskT_all"] = nc.dram_tensor("cmp_maskT_all", [128, NT, M], F16, kind="ExternalInput").ap()
    # per-core
    d["cmp_maskT_own"] = nc.dram_tensor("cmp_maskT_own", [128, 2, M], F16, kind="ExternalInput").ap()
    d["qt16"] = nc.dram_tensor("qt16", [128, 2, HQ, 128], F16, kind="ExternalInput").ap()
    d["qoT"] = nc.dram_tensor("qoT", [128, NT, HPC, 128], F16, kind="ExternalInput").ap()
    d["cw"] = nc.dram_tensor("cw", [128, NT, 3 * HPC], F32, kind="ExternalInput").ap()
    d["slc_keep2"] = nc.dram_tensor("slc_keep2", [128, 2, NB], F32, kind="ExternalInput").ap()
    d["slc_ovr2"] = nc.dram_tensor("slc_ovr2", [128, 2, NB], F32, kind="ExternalInput").ap()
    out_ap = nc.dram_tensor("out", [T, HPC, 128], F32, kind="ExternalOutput").ap()
    if DEBUG:
        for nm, shp, dt_ in [
                ("dbg_keepT", [NB, NT, 128], F16),
                ("dbg_cmpo", [128, NT, HPC, 129], F32),
                ("dbg_swao", [128, NT, HPC, 129], F32),
                ("dbg_pT", [M, HQ, 128], F16),
                ("dbg_slcfin", [128, NB], F32),
                ("dbg_keep01", [128, NB], F32),
                ("dbg_wp5", [128, 5, 256], F16),
                ("dbg_meF5", [128, 5, 128], F16),
                ("dbg_sw5", [128, 5, HPC, 128], F16),
                ("dbg_ktn5", [NB, HPC, 128], F16),
                ("dbg_sp15", [128, 12, 256], F16),
                ("dbg_pa5", [128, 258], F32),
                ("dbg_pa15", [128, 258], F32),
                ("dbg_pa0", [128, 258], F32),
                ("dbg_ot5", [128, HPC, 128], F32)]:
            d[nm] = nc.dram_tensor(nm, shp, dt_, kind="ExternalOutput").ap()

    from contextlib import ExitStack
    with tile.TileContext(nc) as tc:
        with ExitStack() as ctx:
            _body(tc, nc, d, out_ap, ctx)
    nc.compile()
    return nc


def _body(tc, nc, d, out_ap, ctx):
    EXP = mybir.ActivationFunctionType.Exp
    SIG = mybir.ActivationFunctionType.Sigmoid
    MUL = mybir.AluOpType.mult
    ADD = mybir.AluOpType.add
    ISEQ = mybir.AluOpType.is_equal

    const = ctx.enter_context(tc.tile_pool(name="const", bufs=1))
    probs = ctx.enter_context(tc.tile_pool(name="probs", bufs=2))
    winp = ctx.enter_context(tc.tile_pool(name="winp", bufs=1))
    sprob = ctx.enter_context(tc.tile_pool(name="sprob", bufs=2))
    selwp = ctx.enter_context(tc.tile_pool(name="selwp", bufs=2))
    mep = ctx.enter_context(tc.tile_pool(name="mep", bufs=2))
    keepp = ctx.enter_context(tc.tile_pool(name="keepp", bufs=3))
    smallp = ctx.enter_context(tc.tile_pool(name="smallp", bufs=2))
    outp = ctx.enter_context(tc.tile_pool(name="outp", bufs=2))
    scr = ctx.enter_context(tc.tile_pool(name="scr", bufs=2))
    ring = ctx.enter_context(tc.tile_pool(name="ring", bufs=3, space="PSUM"))
    accp = ctx.enter_context(tc.tile_pool(name="accp", bufs=2, space="PSUM"))
    dram = ctx.enter_context(tc.tile_pool(name="dram", bufs=1, space="DRAM"))

    # ---- load constants ----
    kT_sb = const.tile([128, T], F16, tag="kT")
    nc.sync.dma_start(out=kT_sb, in_=d["kT"])
    vext_sb = const.tile([128, NT, 129], F16, tag="vext")
    nc.sync.dma_start(out=vext_sb, in_=d["v_ext"])
    cmpkT_sb = const.tile([128, M], F16, tag="cmpkT")
    nc.sync.dma_start(out=cmpkT_sb, in_=d["cmp_kT"])
    cmprhs_sb = const.tile([M, 162], F16, tag="cmprhs")
    nc.sync.dma_start(out=cmprhs_sb, in_=d["cmp_rhs"])
    e2c_sb = const.tile([NB, NT, 128], F16, tag="e2c")
    nc.sync.dma_start(out=e2c_sb, in_=d["e2c"])
    ident_sb = const.tile([128, 128], F32, tag="ident")
    nc.sync.dma_start(out=ident_sb, in_=d["ident"])
    repI2_sb = const.tile([128, 256], F16, tag="repI2")
    nc.sync.dma_start(out=repI2_sb, in_=d["repI2"])
    repI16_sb = const.tile([128, 2048], F16, tag="repI16")
    nc.sync.dma_start(out=repI16_sb, in_=d["repI16"])
    triNegT_sb = const.tile([128, 128], F16, tag="triNegT")
    nc.sync.dma_start(out=triNegT_sb, in_=d["triNegT"])
    triS01_sb = const.tile([128, 256], F16, tag="triS01")
    nc.sync.dma_start(out=triS01_sb, in_=d["triS01"])
    cmA_sb = const.tile([128, NT, M], F16, tag="cmA")
    nc.sync.dma_start(out=cmA_sb, in_=d["cmp_maskT_all"])
    cmO_sb = const.tile([128, 2, M], F16, tag="cmO")
    nc.sync.dma_start(out=cmO_sb, in_=d["cmp_maskT_own"])
    qt16_sb = const.tile([128, 2, HQ, 128], F16, tag="qt16")
    nc.sync.dma_start(out=qt16_sb, in_=d["qt16"])
    qoT_sb = const.tile([128, NT, HPC, 128], F16, tag="qoT")
    nc.sync.dma_start(out=qoT_sb, in_=d["qoT"])
    cw_sb = const.tile([128, NT, 3 * HPC], F32, tag="cw")
    nc.sync.dma_start(out=cw_sb, in_=d["cw"])
    slck_sb = const.tile([128, 2, NB], F32, tag="slck")
    nc.sync.dma_start(out=slck_sb, in_=d["slc_keep2"])
    slco_sb = const.tile([128, 2, NB], F32, tag="slco")
    nc.sync.dma_start(out=slco_sb, in_=d["slc_ovr2"])
    sg_sb = const.tile([128, NT, 3 * HPC], F32, tag="sg")
    nc.scalar.activation(sg_sb, cw_sb, SIG)

    # flushed per-tile outputs (+denominator col) for cmp / swa streams
    cmpo_sb = const.tile([128, NT, HPC, 129], F32, tag="cmpo")
    swao_sb = const.tile([128, NT, HPC, 129], F32, tag="swao")
    keepT_all = const.tile([NB, NT, 128], F16, tag="keepT_all")

    inb = dram.tile([2, NB, 128], F16, tag="inb")
    outb = dram.tile([NT, NB, 128], F16, tag="outb")

    def emit_chain(mms):
        """mms: list of (bank_key, emit_fn). Call emit_fn(start, stop) with
        per-bank first/last flags (PSUM accumulation-group convention)."""
        first, last = {}, {}
        for i, (b, _) in enumerate(mms):
            first.setdefault(b, i)
            last[b] = i
        for i, (b, fn) in enumerate(mms):
            fn(i == first[b], i == last[b])

    # ================= Phase 1: top-k for own 2 tiles =================
    for ti in range(2):
        pT = probs.tile([M, HQ, 128], F16, tag="pT")
        for half in range(2):
            ps = ring.tile([128, 2, 512], F32, tag="ring", name=f"p1c{ti}_{half}")
            mms = []
            for b in range(2):
                hb = half * 2 + b
                reg = ps[0:M, b, :]
                mms.append((b, lambda s, e, reg=reg, hb=hb: nc.tensor.matmul(
                    reg, lhsT=cmpkT_sb, rhs=qt16_sb[:, ti, 4 * hb:4 * hb + 4, :],
                    start=s, stop=e)))
                mms.append((b, lambda s, e, reg=reg, hb=hb: nc.tensor.matmul(
                    reg, lhsT=cmO_sb[:, ti, :],
                    rhs=repI16_sb[:, 512 * hb:512 * hb + 512], start=s, stop=e)))
            emit_chain(mms)
            nc.scalar.activation(
                pT[:, 8 * half:8 * half + 8, :].rearrange("p h q -> p (h q)"),
                ps[0:M].rearrange("p a b -> p (a b)"), EXP, scale=SCALE)

        # slc numerators + denominators: 16 heads x 33 cols, 8 per bank
        slc_ps = ring.tile([128, 2, 512], F32, tag="ring", name=f"p1s{ti}")
        for h in range(HQ):
            bank, h8 = divmod(h, 8)
            nc.tensor.matmul(slc_ps[:, bank, 33 * h8:33 * h8 + 33],
                             lhsT=pT[0:M, h, :], rhs=cmprhs_sb[:, 0:33],
                             start=(h8 == 0), stop=(h8 == 7))
        slab = smallp.tile([128, 2, 8, 33], F32, tag="slab")
        nc.vector.tensor_copy(
            slab, slc_ps[:, :, 0:264].rearrange("p a (h w) -> p a h w", h=8))
        dn16 = smallp.tile([128, 2, 8], F32, tag="dn16")
        nc.vector.tensor_scalar(dn16, slab[:, :, :, 32], EPS, None, op0=ADD)
        rc16 = smallp.tile([128, 2, 8], F32, tag="rc16")
        nc.vector.reciprocal(rc16, dn16)
        nc.gpsimd.tensor_mul(slab[:, :, :, 0:32], slab[:, :, :, 0:32],
                             rc16.unsqueeze(3).to_broadcast((128, 2, 8, 32)))
        slc_fin = keepp.tile([128, NB], F32, tag="slc_fin")
        nc.vector.tensor_reduce(slc_fin,
                                slab[:, :, :, 0:32].rearrange(
                                    "p a h w -> p w (a h)"),
                                axis=mybir.AxisListType.X, op=ADD)
        nc.gpsimd.tensor_mul(slc_fin, slc_fin, slck_sb[:, ti, :])
        nc.gpsimd.tensor_add(slc_fin, slc_fin, slco_sb[:, ti, :])

        # top-k (proven baseline sequence): zap top 2x8, keep01 = selected
        z1 = keepp.tile([128, NB], F32, tag="z1")
        mx = keepp.tile([128, 8], F32, tag="mx")
        nc.vector.max(mx, slc_fin)
        nc.vector.match_replace(z1, mx, slc_fin, ZAP)
        z2 = keepp.tile([128, NB], F32, tag="z2")
        mx2 = keepp.tile([128, 8], F32, tag="mx2")
        nc.vector.max(mx2, z1)
        nc.vector.match_replace(z2, mx2, z1, ZAP)
        keep01 = keepp.tile([128, NB], F32, tag="keep01")
        nc.vector.tensor_tensor(keep01, slc_fin, z2, op=ISEQ)  # 1 = NOT selected
        nc.vector.tensor_scalar(keep01, keep01, -1.0, 1.0, op0=MUL, op1=ADD)
        if DEBUG and ti == 0:
            nc.sync.dma_start(out=d["dbg_pT"], in_=pT)
            nc.sync.dma_start(out=d["dbg_slcfin"], in_=slc_fin)
            nc.sync.dma_start(out=d["dbg_keep01"], in_=keep01)

        ps_t = ring.tile([128, 2, 512], F32, tag="ring", name=f"p1t{ti}")
        nc.tensor.transpose(ps_t[0:NB, 0, 0:128], keep01, ident_sb)
        kpt = keepp.tile([NB, 128], F16, tag="kpt")
        nc.vector.tensor_copy(kpt, ps_t[0:NB, 0, 0:128])
        nc.sync.dma_start(out=inb[ti], in_=kpt)

    # ---- AllGather keep masks: [2,NB,128] x8 -> [NT,NB,128] ----
    nc.gpsimd.collective_compute(
        "AllGather", mybir.AluOpType.bypass,
        replica_groups=[list(range(NCORES))],
        ins=[inb.opt()], outs=[outb.opt()])
    nc.sync.dma_start(out=keepT_all, in_=outb.rearrange("t b q -> b t q"))
    if DEBUG:
        nc.sync.dma_start(out=d["dbg_keepT"], in_=keepT_all)

    # ================= Phase 2a: cmp attention, own heads, all tiles =====
    for tq in range(0, NT, 4):
        ps = ring.tile([128, 2, 512], F32, tag="ring", name=f"p2c{tq}")
        mms = []
        for i in range(4):
            t = tq + i
            reg = ps[0:M, i // 2, 256 * (i % 2):256 * (i % 2) + 256]
            mms.append((i // 2, lambda s, e, reg=reg, t=t: nc.tensor.matmul(
                reg, lhsT=cmpkT_sb, rhs=qoT_sb[:, t], start=s, stop=e)))
            mms.append((i // 2, lambda s, e, reg=reg, t=t: nc.tensor.matmul(
                reg, lhsT=cmA_sb[:, t, :], rhs=repI2_sb, start=s, stop=e)))
        emit_chain(mms)
        pTo = probs.tile([M, 4, 256], F16, tag="pTo")
        nc.scalar.activation(pTo.rearrange("p a b -> p (a b)"),
                             ps[0:M].rearrange("p a b -> p (a b)"),
                             EXP, scale=SCALE)
        for i in range(4):
            t = tq + i
            pa = accp.tile([128, 512], F32, tag="acc", name=f"p2a{t}")
            for h in range(HPC):
                nc.tensor.matmul(pa[:, 129 * h:129 * h + 129],
                                 lhsT=pTo[0:M, i, 128 * h:128 * h + 128],
                                 rhs=cmprhs_sb[:, 33:162],
                                 start=(h == 0), stop=(h == 1))
            # flush; +EPS on the DENOMINATOR only so all-masked tile-0
            # queries give cmp_o = 0/EPS = 0 (values must stay exact 0)
            pav = pa[:, 0:258].rearrange("p (h w) -> p h w", h=HPC)
            nc.vector.tensor_copy(cmpo_sb[:, t, :, 0:128], pav[:, :, 0:128])
            nc.vector.tensor_scalar(cmpo_sb[:, t, :, 128], pav[:, :, 128],
                                    EPS, None, op0=ADD)

    # ================= Phase 2b: swa attention, own heads, all tiles =====
    wps = {}
    for t in range(NT):
        c_lo = max(0, t - WCH)
        nw = t - c_lo + 1  # 1..5 window chunks
        wp = winp.tile([128, 5, 256], F16, tag=f"wp{t}")
        wps[t] = wp
        n0 = min(nw, 4)
        psA = ring.tile([128, 2, 512], F32, tag="ring", name=f"p2wA{t}")
        mms = []
        for wi in range(n0):
            c = c_lo + wi
            reg = psA[:, wi // 2, 256 * (wi % 2):256 * (wi % 2) + 256]
            mms.append((wi // 2, lambda s, e, reg=reg, c=c: nc.tensor.matmul(
                reg, lhsT=kT_sb[:, 128 * c:128 * c + 128], rhs=qoT_sb[:, t],
                start=s, stop=e)))
            if c == t:  # diag: fold causal tri
                mms.append((wi // 2, lambda s, e, reg=reg: nc.tensor.matmul(
                    reg, lhsT=triNegT_sb, rhs=repI2_sb, start=s, stop=e)))
        emit_chain(mms)
        nc.scalar.activation(
            wp[:, 0:n0, :].rearrange("p a b -> p (a b)"),
            psA[:, :, :].rearrange("p a b -> p (a b)")[:, 0:n0 * 256],
            EXP, scale=SCALE)
        if nw == 5:
            psB = ring.tile([128, 2, 512], F32, tag="ring", name=f"p2wB{t}")
            reg = psB[:, 0, 0:256]
            nc.tensor.matmul(reg, lhsT=kT_sb[:, 128 * t:128 * t + 128],
                             rhs=qoT_sb[:, t], start=True, stop=False)
            nc.tensor.matmul(reg, lhsT=triNegT_sb, rhs=repI2_sb,
                             start=False, stop=True)
            nc.scalar.activation(wp[:, 4, :], reg, EXP, scale=SCALE)
        # window-start partial chunk: keep keys with k_local > q_local
        sw0 = None
        if t >= WCH:
            sw0 = scr.tile([128, 256], F16, tag="sw0")
            nc.gpsimd.tensor_mul(sw0, wp[:, 0, :], triS01_sb)
        pa = accp.tile([128, 512], F32, tag="acc", name=f"p2w{t}")
        for wi in range(nw):
            c = c_lo + wi
            for h in range(HPC):
                src = (sw0[:, 128 * h:128 * h + 128] if (wi == 0 and t >= WCH)
                       else wp[:, wi, 128 * h:128 * h + 128])
                nc.tensor.matmul(pa[:, 129 * h:129 * h + 129], lhsT=src,
                                 rhs=vext_sb[:, c, :],
                                 start=(wi == 0 and h == 0),
                                 stop=(wi == nw - 1 and h == HPC - 1))
        nc.vector.tensor_copy(swao_sb[:, t].rearrange("p h w -> p (h w)"),
                              pa[:, 0:258])
        if DEBUG and t == 5:
            nc.sync.dma_start(out=d["dbg_wp5"], in_=wp)

    # ================= Phase 3: sel attention + combine, all tiles =======
    for t in range(NT):
        c_lo = max(0, t - WCH)
        nw = t - c_lo + 1
        ns = c_lo  # sel-only chunks 0..c_lo-1

        ktn = keepp.tile([NB, HPC, 128], F16, tag="ktn")
        for h in range(HPC):
            nc.vector.tensor_scalar(ktn[:, h, :], keepT_all[:, t, :],
                                    -1.0, MASKNEG, op0=ADD, op1=MUL)

        # exact {0,1} keep mask per (key, query) for the window chunks
        me_ps = ring.tile([128, 2, 512], F32, tag="ring", name=f"p3m{t}")
        mms = []
        for wi in range(nw):
            c = c_lo + wi
            reg = me_ps[:, wi // 4, 128 * (wi % 4):128 * (wi % 4) + 128]
            mms.append((wi // 4, lambda s, e, reg=reg, c=c: nc.tensor.matmul(
                reg, lhsT=e2c_sb[:, c, :], rhs=keepT_all[:, t, :],
                start=s, stop=e)))
        emit_chain(mms)
        meF = mep.tile([128, 5, 128], F16, tag="meF")
        n0 = min(nw, 4)
        nc.vector.tensor_copy(
            meF[:, 0:n0, :].rearrange("p a b -> p (a b)"),
            me_ps[:, 0, 0:n0 * 128])
        if nw == 5:
            nc.vector.tensor_copy(meF[:, 4, :], me_ps[:, 1, 0:128])

        sw = selwp.tile([128, 5, HPC, 128], F16, tag="sw")
        nc.gpsimd.tensor_mul(
            sw[:, 0:nw],
            wps[t][:, 0:nw].rearrange("p w (h q) -> p w h q", h=HPC),
            meF[:, 0:nw].unsqueeze(2).to_broadcast((128, nw, HPC, 128)))
        if DEBUG and t == 5:
            nc.sync.dma_start(out=d["dbg_meF5"], in_=meF)
            nc.sync.dma_start(out=d["dbg_sw5"], in_=sw)
            nc.sync.dma_start(out=d["dbg_ktn5"], in_=ktn)

        sp = None
        if ns > 0:
            sp = sprob.tile([128, 12, 256], F16, tag="sp")
            for cq in range(0, ns, 4):
                n = min(4, ns - cq)
                ps = ring.tile([128, 2, 512], F32, tag="ring",
                               name=f"p3s{t}_{cq}")
                mms = []
                for i in range(n):
                    c = cq + i
                    reg = ps[:, i // 2, 256 * (i % 2):256 * (i % 2) + 256]
                    mms.append((i // 2, lambda s, e, reg=reg, c=c:
                                nc.tensor.matmul(
                        reg, lhsT=kT_sb[:, 128 * c:128 * c + 128],
                        rhs=qoT_sb[:, t], start=s, stop=e)))
                    mms.append((i // 2, lambda s, e, reg=reg, c=c:
                                nc.tensor.matmul(
                        reg, lhsT=e2c_sb[:, c, :], rhs=ktn, start=s, stop=e)))
                emit_chain(mms)
                nc.scalar.activation(
                    sp[:, cq:cq + n, :].rearrange("p a b -> p (a b)"),
                    ps[:, :, :].rearrange("p a b -> p (a b)")[:, 0:n * 256],
                    EXP, scale=SCALE)
            if DEBUG and t == 15:
                nc.sync.dma_start(out=d["dbg_sp15"][:, 0:11], in_=sp[:, 0:11])

        pa = accp.tile([128, 512], F32, tag="acc", name=f"p3a{t}")
        for c in range(t + 1):
            for h in range(HPC):
                if c < c_lo:
                    src = sp[:, c, 128 * h:128 * h + 128]
                else:
                    src = sw[:, c - c_lo, h, :]
                nc.tensor.matmul(pa[:, 129 * h:129 * h + 129], lhsT=src,
                                 rhs=vext_sb[:, c, :],
                                 start=(c == 0 and h == 0),
                                 stop=(c == t and h == HPC - 1))

        if DEBUG and t in (0, 5, 15):
            stg = outp.tile([128, 258], F32, tag="dbgstg", name=f"stg{t}")
            nc.vector.tensor_copy(stg, pa[:, 0:258])
            nc.sync.dma_start(out=d[f"dbg_pa{t}"], in_=stg)

        # ---- combine ----
        ot = outp.tile([128, HPC, 128], F32, tag="ot")
        for h in range(HPC):
            rc0 = smallp.tile([128, 1], F32, tag="rc0")
            nc.vector.reciprocal(rc0, cmpo_sb[:, t, h, 128:129])
            rc1 = smallp.tile([128, 1], F32, tag="rc1")
            nc.vector.reciprocal(rc1, pa[:, 129 * h + 128:129 * h + 129])
            rc2 = smallp.tile([128, 1], F32, tag="rc2")
            nc.vector.reciprocal(rc2, swao_sb[:, t, h, 128:129])
            rs0 = smallp.tile([128, 1], F32, tag="rs0")
            nc.vector.tensor_mul(rs0, rc0, sg_sb[:, t, 3 * h:3 * h + 1])
            rs1 = smallp.tile([128, 1], F32, tag="rs1")
            nc.vector.tensor_mul(rs1, rc1, sg_sb[:, t, 3 * h + 1:3 * h + 2])
            rs2 = smallp.tile([128, 1], F32, tag="rs2")
            nc.vector.tensor_mul(rs2, rc2, sg_sb[:, t, 3 * h + 2:3 * h + 3])
            acc_a = outp.tile([128, 128], F32, tag="acc_a")
            nc.vector.tensor_scalar(acc_a, cmpo_sb[:, t, h, 0:128], rs0,
                                    None, op0=MUL)
            acc_b = outp.tile([128, 128], F32, tag="acc_b")
            nc.vector.scalar_tensor_tensor(
                acc_b, pa[:, 129 * h:129 * h + 128], rs1, acc_a,
                op0=MUL, op1=ADD)
            nc.vector.scalar_tensor_tensor(
                ot[:, h, :], swao_sb[:, t, h, 0:128], rs2, acc_b,
                op0=MUL, op1=ADD)
        nc.sync.dma_start(out=out_ap[t * 128:(t + 1) * 128], in_=ot)
        if DEBUG and t == 5:
            nc.sync.dma_start(out=d["dbg_ot5"], in_=ot)

    if DEBUG:
        nc.sync.dma_start(out=d["dbg_cmpo"], in_=cmpo_sb)
        nc.sync.dma_start(out=d["dbg_swao"], in_=swao_sb)


def _host_inputs(q, k, v, cw):
    """Precompute per-core input arrays (numpy only; no FLOP-heavy work)."""
    q = np.ascontiguousarray(q, np.float32)
    k2 = np.ascontiguousarray(k[:, 0, :], np.float32)
    v2 = np.ascontiguousarray(v[:, 0, :], np.float32)
    cw = np.ascontiguousarray(cw, np.float32)
    f16 = np.float16

    kT = np.ascontiguousarray(k2.T).astype(f16)
    v_ext = np.ascontiguousarray(
        np.concatenate([v2, np.ones((T, 1), np.float32)], 1)
        .reshape(NT, 128, 129).transpose(1, 0, 2)).astype(f16)
    idx = np.arange(M)[:, None] * ST + np.arange(KS)[None, :]
    cmp_k = k2[idx].mean(1)
    cmp_v = v2[idx].mean(1)
    cmp_kT = np.ascontiguousarray(cmp_k.T).astype(f16)
    ju, bu = KS // ST, BS // ST
    j = np.arange(M)[:, None]
    b = np.arange(NB)[None, :]
    ov = np.maximum(0, np.minimum(j + ju, (b + 1) * bu) - np.maximum(j, b * bu))
    Wmap = (ov / ju).astype(np.float32)
    cmp_rhs = np.ascontiguousarray(
        np.concatenate([Wmap, np.ones((M, 1), np.float32), cmp_v,
                        np.ones((M, 1), np.float32)], 1)).astype(f16)

    t_pos = np.arange(T)
    ar = np.arange(128)
    c_idx = np.arange(NT)
    bidx = np.arange(NB)
    e2c = np.ascontiguousarray(
        (bidx[:, None, None] ==
         (2 * c_idx[None, :, None] + ar[None, None, :] // 64)).astype(f16))
    ident = np.eye(128, dtype=np.float32)
    eye16 = np.eye(128, dtype=np.float32).astype(f16)
    repI2 = np.ascontiguousarray(np.tile(eye16, (1, 2)))
    repI16 = np.ascontiguousarray(np.tile(eye16, (1, 16)))
    # triNegT[r, key] = -MASKNEG where key > r (future within diag chunk)
    triNegT = np.ascontiguousarray(
        (-MASKNEG * (ar[None, :] > ar[:, None])).astype(f16))
    # triS01[k, h*128+q] = 1 where k > q (valid at window-start chunk)
    triS01 = np.ascontiguousarray(
        np.tile((ar[:, None] > ar[None, :]).astype(np.float32), (1, 2))
        .astype(f16))
    # cmp validity: keep[m, t] = (m*ST + KS <= t+1); maskT[q, t, m]
    keep_cmp = ((np.arange(M)[:, None] * ST + KS) <= (t_pos[None, :] + 1))
    cmp_maskT_all = np.ascontiguousarray(
        (-MASKNEG * (1.0 - keep_cmp.astype(np.float32)))
        .T.reshape(NT, 128, M).transpose(1, 0, 2)).astype(f16)

    cur_blk = t_pos // BS
    future = bidx[None, :] > cur_blk[:, None]
    forced = (bidx[None, :] < NINIT) | (
        (bidx[None, :] <= cur_blk[:, None])
        & (bidx[None, :] > cur_blk[:, None] - NLOCAL))
    slc_keep = np.ascontiguousarray(
        (~(future | forced)).astype(np.float32)
        .reshape(NT, 128, NB).transpose(1, 0, 2))
    slc_ovr = np.ascontiguousarray(
        np.where(forced, BIGC, np.where(future, NEGC, 0.0)).astype(np.float32)
        .reshape(NT, 128, NB).transpose(1, 0, 2))

    qt_all = q.reshape(NT, 128, HQ, D).transpose(3, 0, 2, 1)  # [D, t, h, q]

    shared = dict(kT=kT, v_ext=v_ext, cmp_kT=cmp_kT, cmp_rhs=cmp_rhs,
                  e2c=e2c, ident=ident, repI2=repI2, repI16=repI16,
                  triNegT=triNegT, triS01=triS01,
                  cmp_maskT_all=cmp_maskT_all)
    in_maps = []
    for c in range(NCORES):
        hs = slice(c * HPC, (c + 1) * HPC)
        ts = slice(2 * c, 2 * c + 2)
        qt16 = np.ascontiguousarray(
            qt_all[:, ts].transpose(0, 1, 2, 3)).astype(f16)  # [D, 2, h, q]
        qoT = np.ascontiguousarray(qt_all[:, :, hs, :]).astype(f16)
        cw_own = np.ascontiguousarray(
            cw[:, hs, :].reshape(NT, 128, 3 * HPC).transpose(1, 0, 2))
        in_maps.append(dict(
            shared, qt16=qt16, qoT=qoT, cw=cw_own,
            cmp_maskT_own=np.ascontiguousarray(cmp_maskT_all[:, ts, :]),
            slc_keep2=np.ascontiguousarray(slc_keep[:, ts, :]),
            slc_ovr2=np.ascontiguousarray(slc_ovr[:, ts, :])))
    return in_maps


_PROGRAM = None


def _get_program():
    global _PROGRAM
    if _PROGRAM is None:
        _PROGRAM = _build_program()
    return _PROGRAM


def kernel(q, k, v, combine_weight, cu_seqlens, _trace=False):
    nc = _get_program()
    in_maps = _host_inputs(np.asarray(q), np.asarray(k), np.asarray(v),
                           np.asarray(combine_weight))
    res = run_bass_kernel_spmd(nc, in_maps, core_ids=list(range(NCORES)),
                               trace=_trace)
    outs = [res.results[c]["out"] for c in range(NCORES)]
    full = np.concatenate(outs, axis=1).astype(np.float32)
    if _trace:
        kernel._last_results = res
    return full


# revision 23
# speedup vs baseline: 1.7771x; 1.0094x over previous
"""NSA (native sparse attention) forward kernel for Trainium2, 8 NeuronCores.

v3 design (vs v2 baseline at 288us):
- Head-TP for sel/swa attention (core c owns heads {2c, 2c+1}) -- keeps the
  program SPMD-uniform.
- The cmp/top-k pipeline (which needs all 16 heads) is SHARDED BY QUERY TILE
  (core c computes top-k for tiles {2c, 2c+1} only, 8x less work than the v2
  all-tiles-on-every-core scheme) and the tiny keep masks (16KB/core) are
  AllGather'd across the 8 cores. The collective latency hides under the
  top-k-independent swa + cmp-own-head work.
- Window chunks share ONE set of raw scores between sel and swa: exp(raw)
  probs are kept in SBUF; sel probs = probs * keep-mask (gpsimd multiply with
  an exact {0,1} mask built by a tiny PE matmul), swa start-chunk probs =
  probs * anti-tri constant. This kills the separate swa score matmuls and
  all per-chunk mask matmuls on window chunks.
- Causal diag mask and cmp validity masks folded into the PE score
  accumulation (constant -MASKNEG matmuls) instead of gpsimd multiplies.
- Activations batched 4 chunks wide ([128,1024]+) to amortize the 352-cycle
  ACT fixed overhead (v2 paid it per 256-wide slot).
- PSUM: ring of 2-bank score tiles (bufs=3) + 1-bank PV accumulators
  (bufs=2) = 8 banks exactly.
"""

import numpy as np

import concourse.bacc as bacc
import concourse.bass as bass
import concourse.tile as tile
from concourse import mybir
from concourse.bass_utils import run_bass_kernel_spmd

# ---- problem constants (hardcoded per contest rules) ----
T, HQ, D = 2048, 16, 128
KS, ST, BS = 32, 16, 64
TOP_N, NINIT, NLOCAL, WIN = 16, 2, 1, 512
M = (T - KS) // ST + 1          # 127 compressed tokens
NB = T // BS                    # 32 selection blocks
NT = T // 128                   # 16 query tiles
NCORES = 8
HPC = HQ // NCORES              # 2 heads per core
SCALE = float(D) ** -0.5
NEGC = -1e30
BIGC = 1e30
EPS = 1e-30
ZAP = -1e38
MASKNEG = 400.0                 # -(1-keep)*400: exp underflows fp16 to exact 0
F32 = mybir.dt.float32
F16 = mybir.dt.float16

WCH = WIN // 128                # window spans 4 full chunks + diag


import os
DEBUG = bool(os.environ.get("KDBG"))


def _build_program():
    nc = bacc.Bacc("TRN2", target_bir_lowering=False, debug=False,
                   num_devices=NCORES)

    d = {}
    # shared constants
    d["kT"] = nc.dram_tensor("kT", [128, T], F16, kind="ExternalInput").ap()
    d["v_ext"] = nc.dram_tensor("v_ext", [128, NT, 129], F16, kind="ExternalInput").ap()
    d["cmp_kT"] = nc.dram_tensor("cmp_kT", [128, M], F16, kind="ExternalInput").ap()
    d["cmp_rhs"] = nc.dram_tensor("cmp_rhs", [M, 162], F16, kind="ExternalInput").ap()
    d["e2c"] = nc.dram_tensor("e2c", [NB, NT, 128], F16, kind="ExternalInput").ap()
    d["ident"] = nc.dram_tensor("ident", [128, 128], F32, kind="ExternalInput").ap()
    d["repI2"] = nc.dram_tensor("repI2", [128, 256], F16, kind="ExternalInput").ap()
    d["repI16"] = nc.dram_tensor("repI16", [128, 2048], F16, kind="ExternalInput").ap()
    d["triNegT"] = nc.dram_tensor("triNegT", [128, 128], F16, kind="ExternalInput").ap()
    d["triS01"] = nc.dram_tensor("triS01", [128, 256], F16, kind="ExternalInput").ap()
    d["cmp_maskT_all"] = nc.dram_tensor("cmp_maskT_all", [128, NT, M], F16, kind="ExternalInput").ap()
    # per-core
    d["cmp_maskT_own"] = nc.dram_tensor("cmp_maskT_own", [128, 2, M], F16, kind="ExternalInput").ap()
    d["qt16"] = nc.dram_tensor("qt16", [128, 2, HQ, 128], F16, kind="ExternalInput").ap()
    d["qoT"] = nc.dram_tensor("qoT", [128, NT, HPC, 128], F16, kind="ExternalInput").ap()
    d["cw"] = nc.dram_tensor("cw", [128, NT, 3 * HPC], F32, kind="ExternalInput").ap()
    d["slc_keep2"] = nc.dram_tensor("slc_keep2", [128, 2, NB], F32, kind="ExternalInput").ap()
    d["slc_ovr2"] = nc.dram_tensor("slc_ovr2", [128, 2, NB], F32, kind="ExternalInput").ap()
    out_ap = nc.dram_tensor("out", [T, HPC, 128], F32, kind="ExternalOutput").ap()
    if DEBUG:
        for nm, shp, dt_ in [
                ("dbg_keepT", [NB, NT, 128], F16),
                ("dbg_cmpo", [128, NT, HPC, 129], F32),
                ("dbg_swao", [128, NT, HPC, 129], F32),
                ("dbg_pT", [M, HQ, 128], F16),
                ("dbg_slcfin", [128, NB], F32),
                ("dbg_keep01", [128, NB], F32),
                ("dbg_wp5", [128, 5, 256], F16),
                ("dbg_meF5", [128, 5, 128], F16),
                ("dbg_sw5", [128, 5, HPC, 128], F16),
                ("dbg_ktn5", [NB, HPC, 128], F16),
                ("dbg_sp15", [128, 12, 256], F16),
                ("dbg_pa5", [128, 258], F32),
                ("dbg_pa15", [128, 258], F32),
                ("dbg_pa0", [128, 258], F32),
                ("dbg_ot5", [128, HPC, 128], F32)]:
            d[nm] = nc.dram_tensor(nm, shp, dt_, kind="ExternalOutput").ap()

    from contextlib import ExitStack
    with tile.TileContext(nc) as tc:
        with ExitStack() as ctx:
            _body(tc, nc, d, out_ap, ctx)
    nc.compile()
    return nc


def _body(tc, nc, d, out_ap, ctx):
    EXP = mybir.ActivationFunctionType.Exp
    SIG = mybir.ActivationFunctionType.Sigmoid
    MUL = mybir.AluOpType.mult
    ADD = mybir.AluOpType.add
    ISEQ = mybir.AluOpType.is_equal

    const = ctx.enter_context(tc.tile_pool(name="const", bufs=1))
    probs = ctx.enter_context(tc.tile_pool(name="probs", bufs=2))
    winp = ctx.enter_context(tc.tile_pool(name="winp", bufs=1))
    sprob = ctx.enter_context(tc.tile_pool(name="sprob", bufs=2))
    selwp = ctx.enter_context(tc.tile_pool(name="selwp", bufs=2))
    mep = ctx.enter_context(tc.tile_pool(name="mep", bufs=2))
    keepp = ctx.enter_context(tc.tile_pool(name="keepp", bufs=3))
    smallp = ctx.enter_context(tc.tile_pool(name="smallp", bufs=2))
    outp = ctx.enter_context(tc.tile_pool(name="outp", bufs=2))
    scr = ctx.enter_context(tc.tile_pool(name="scr", bufs=2))
    ring = ctx.enter_context(tc.tile_pool(name="ring", bufs=3, space="PSUM"))
    accp = ctx.enter_context(tc.tile_pool(name="accp", bufs=2, space="PSUM"))
    dram = ctx.enter_context(tc.tile_pool(name="dram", bufs=1, space="DRAM"))

    # ---- load constants ----
    kT_sb = const.tile([128, T], F16, tag="kT")
    nc.sync.dma_start(out=kT_sb, in_=d["kT"])
    vext_sb = const.tile([128, NT, 129], F16, tag="vext")
    nc.sync.dma_start(out=vext_sb, in_=d["v_ext"])
    cmpkT_sb = const.tile([128, M], F16, tag="cmpkT")
    nc.sync.dma_start(out=cmpkT_sb, in_=d["cmp_kT"])
    cmprhs_sb = const.tile([M, 162], F16, tag="cmprhs")
    nc.sync.dma_start(out=cmprhs_sb, in_=d["cmp_rhs"])
    e2c_sb = const.tile([NB, NT, 128], F16, tag="e2c")
    nc.sync.dma_start(out=e2c_sb, in_=d["e2c"])
    ident_sb = const.tile([128, 128], F32, tag="ident")
    nc.sync.dma_start(out=ident_sb, in_=d["ident"])
    repI2_sb = const.tile([128, 256], F16, tag="repI2")
    nc.sync.dma_start(out=repI2_sb, in_=d["repI2"])
    repI16_sb = const.tile([128, 2048], F16, tag="repI16")
    nc.sync.dma_start(out=repI16_sb, in_=d["repI16"])
    triNegT_sb = const.tile([128, 128], F16, tag="triNegT")
    nc.sync.dma_start(out=triNegT_sb, in_=d["triNegT"])
    triS01_sb = const.tile([128, 256], F16, tag="triS01")
    nc.sync.dma_start(out=triS01_sb, in_=d["triS01"])
    cmA_sb = const.tile([128, NT, M], F16, tag="cmA")
    nc.sync.dma_start(out=cmA_sb, in_=d["cmp_maskT_all"])
    cmO_sb = const.tile([128, 2, M], F16, tag="cmO")
    nc.sync.dma_start(out=cmO_sb, in_=d["cmp_maskT_own"])
    qt16_sb = const.tile([128, 2, HQ, 128], F16, tag="qt16")
    nc.sync.dma_start(out=qt16_sb, in_=d["qt16"])
    qoT_sb = const.tile([128, NT, HPC, 128], F16, tag="qoT")
    nc.sync.dma_start(out=qoT_sb, in_=d["qoT"])
    cw_sb = const.tile([128, NT, 3 * HPC], F32, tag="cw")
    nc.sync.dma_start(out=cw_sb, in_=d["cw"])
    slck_sb = const.tile([128, 2, NB], F32, tag="slck")
    nc.sync.dma_start(out=slck_sb, in_=d["slc_keep2"])
    slco_sb = const.tile([128, 2, NB], F32, tag="slco")
    nc.sync.dma_start(out=slco_sb, in_=d["slc_ovr2"])
    sg_sb = const.tile([128, NT, 3 * HPC], F32, tag="sg")
    nc.scalar.activation(sg_sb, cw_sb, SIG)

    # flushed per-tile outputs (+denominator col) for cmp / swa streams
    cmpo_sb = const.tile([128, NT, HPC, 129], F32, tag="cmpo")
    swao_sb = const.tile([128, NT, HPC, 129], F32, tag="swao")
    keepT_all = const.tile([NB, NT, 128], F16, tag="keepT_all")

    inb = dram.tile([2, NB, 128], F16, tag="inb")
    outb = dram.tile([NT, NB, 128], F16, tag="outb")

    def emit_chain(mms):
        """mms: list of (bank_key, emit_fn). Call emit_fn(start, stop) with
        per-bank first/last flags (PSUM accumulation-group convention)."""
        first, last = {}, {}
        for i, (b, _) in enumerate(mms):
            first.setdefault(b, i)
            last[b] = i
        for i, (b, fn) in enumerate(mms):
            fn(i == first[b], i == last[b])

    # ================= Phase 1: top-k for own 2 tiles =================
    keep01s = []
    for ti in range(2):
        pT = probs.tile([M, HQ, 128], F16, tag="pT")
        for half in range(2):
            ps = ring.tile([128, 2, 512], F32, tag="ring", name=f"p1c{ti}_{half}")
            mms = []
            for b in range(2):
                hb = half * 2 + b
                reg = ps[0:M, b, :]
                mms.append((b, lambda s, e, reg=reg, hb=hb: nc.tensor.matmul(
                    reg, lhsT=cmpkT_sb, rhs=qt16_sb[:, ti, 4 * hb:4 * hb + 4, :],
                    start=s, stop=e)))
                mms.append((b, lambda s, e, reg=reg, hb=hb: nc.tensor.matmul(
                    reg, lhsT=cmO_sb[:, ti, :],
                    rhs=repI16_sb[:, 512 * hb:512 * hb + 512], start=s, stop=e)))
            emit_chain(mms)
            nc.scalar.activation(
                pT[:, 8 * half:8 * half + 8, :].rearrange("p h q -> p (h q)"),
                ps[0:M].rearrange("p a b -> p (a b)"), EXP, scale=SCALE)

        # slc numerators + denominators: 16 heads x 33 cols, 8 per bank
        slc_ps = ring.tile([128, 2, 512], F32, tag="ring", name=f"p1s{ti}")
        for h in range(HQ):
            bank, h8 = divmod(h, 8)
            nc.tensor.matmul(slc_ps[:, bank, 33 * h8:33 * h8 + 33],
                             lhsT=pT[0:M, h, :], rhs=cmprhs_sb[:, 0:33],
                             start=(h8 == 0), stop=(h8 == 7))
        slab = smallp.tile([128, 2, 8, 33], F32, tag="slab")
        nc.vector.tensor_copy(
            slab, slc_ps[:, :, 0:264].rearrange("p a (h w) -> p a h w", h=8))
        dn16 = smallp.tile([128, 2, 8], F32, tag="dn16")
        nc.vector.tensor_scalar(dn16, slab[:, :, :, 32], EPS, None, op0=ADD)
        rc16 = smallp.tile([128, 2, 8], F32, tag="rc16")
        nc.vector.reciprocal(rc16, dn16)
        nc.gpsimd.tensor_mul(slab[:, :, :, 0:32], slab[:, :, :, 0:32],
                             rc16.unsqueeze(3).to_broadcast((128, 2, 8, 32)))
        slc_fin = keepp.tile([128, NB], F32, tag="slc_fin")
        nc.vector.tensor_reduce(slc_fin,
                                slab[:, :, :, 0:32].rearrange(
                                    "p a h w -> p w (a h)"),
                                axis=mybir.AxisListType.X, op=ADD)
        nc.gpsimd.tensor_mul(slc_fin, slc_fin, slck_sb[:, ti, :])
        nc.gpsimd.tensor_add(slc_fin, slc_fin, slco_sb[:, ti, :])

        # top-k (proven baseline sequence): zap top 2x8, keep01 = selected
        z1 = keepp.tile([128, NB], F32, tag="z1")
        mx = keepp.tile([128, 8], F32, tag="mx")
        nc.vector.max(mx, slc_fin)
        nc.vector.match_replace(z1, mx, slc_fin, ZAP)
        z2 = keepp.tile([128, NB], F32, tag="z2")
        mx2 = keepp.tile([128, 8], F32, tag="mx2")
        nc.vector.max(mx2, z1)
        nc.vector.match_replace(z2, mx2, z1, ZAP)
        keep01 = keepp.tile([128, NB], F32, tag="keep01")
        nc.vector.tensor_tensor(keep01, slc_fin, z2, op=ISEQ)  # 1 = NOT selected
        nc.vector.tensor_scalar(keep01, keep01, -1.0, 1.0, op0=MUL, op1=ADD)
        keep01s.append(keep01)
        if DEBUG and ti == 0:
            nc.sync.dma_start(out=d["dbg_pT"], in_=pT)
            nc.sync.dma_start(out=d["dbg_slcfin"], in_=slc_fin)
            nc.sync.dma_start(out=d["dbg_keep01"], in_=keep01)

    # transpose + bounce-DMA emitted here so they sit BEHIND phase 2a's
    # matmuls in the PE queue (no head-of-line stall on the top-k chain);
    # the collective still fires as soon as both kpts land.
    for ti in range(2):
        ps_t = ring.tile([128, 2, 512], F32, tag="ring", name=f"p1t{ti}")
        nc.tensor.transpose(ps_t[0:NB, 0, 0:128], keep01s[ti], ident_sb)
        kpt = keepp.tile([NB, 128], F16, tag="kpt")
        nc.vector.tensor_copy(kpt, ps_t[0:NB, 0, 0:128])
        nc.sync.dma_start(out=inb[ti], in_=kpt)

    # ---- AllGather keep masks: [2,NB,128] x8 -> [NT,NB,128] ----
    nc.gpsimd.collective_compute(
        "AllGather", mybir.AluOpType.bypass,
        replica_groups=[list(range(NCORES))],
        ins=[inb.opt()], outs=[outb.opt()])
    nc.sync.dma_start(out=keepT_all, in_=outb.rearrange("t b q -> b t q"))
    if DEBUG:
        nc.sync.dma_start(out=d["dbg_keepT"], in_=keepT_all)
    # (keep-1)*MASKNEG for every tile, both head copies, one op per head
    ktn_all = const.tile([NB, NT, HPC, 128], F16, tag="ktn_all")
    for h in range(HPC):
        nc.vector.tensor_scalar(ktn_all[:, :, h, :], keepT_all,
                                -1.0, MASKNEG, op0=ADD, op1=MUL)

    # ================= Phase 2a: cmp attention, own heads, all tiles =====
    for tq in range(0, NT, 4):
        ps = ring.tile([128, 2, 512], F32, tag="ring", name=f"p2c{tq}")
        mms = []
        for i in range(4):
            t = tq + i
            reg = ps[0:M, i // 2, 256 * (i % 2):256 * (i % 2) + 256]
            mms.append((i // 2, lambda s, e, reg=reg, t=t: nc.tensor.matmul(
                reg, lhsT=cmpkT_sb, rhs=qoT_sb[:, t], start=s, stop=e)))
            mms.append((i // 2, lambda s, e, reg=reg, t=t: nc.tensor.matmul(
                reg, lhsT=cmA_sb[:, t, :], rhs=repI2_sb, start=s, stop=e)))
        emit_chain(mms)
        pTo = probs.tile([M, 4, 256], F16, tag="pTo")
        nc.scalar.activation(pTo.rearrange("p a b -> p (a b)"),
                             ps[0:M].rearrange("p a b -> p (a b)"),
                             EXP, scale=SCALE)
        for i in range(4):
            t = tq + i
            pa = accp.tile([128, 512], F32, tag="acc", name=f"p2a{t}")
            for h in range(HPC):
                nc.tensor.matmul(pa[:, 129 * h:129 * h + 129],
                                 lhsT=pTo[0:M, i, 128 * h:128 * h + 128],
                                 rhs=cmprhs_sb[:, 33:162],
                                 start=(h == 0), stop=(h == 1))
            # flush; +EPS on the DENOMINATOR only so all-masked tile-0
            # queries give cmp_o = 0/EPS = 0 (values must stay exact 0)
            pav = pa[:, 0:258].rearrange("p (h w) -> p h w", h=HPC)
            nc.vector.tensor_copy(cmpo_sb[:, t, :, 0:128], pav[:, :, 0:128])
            nc.vector.tensor_scalar(cmpo_sb[:, t, :, 128], pav[:, :, 128],
                                    EPS, None, op0=ADD)

    # ================= Phase 2b: swa attention, own heads, all tiles =====
    wps = {}
    for t in range(NT):
        c_lo = max(0, t - WCH)
        nw = t - c_lo + 1  # 1..5 window chunks
        wp = winp.tile([128, 5, 256], F16, tag=f"wp{t}")
        wps[t] = wp
        n0 = min(nw, 4)
        psA = ring.tile([128, 2, 512], F32, tag="ring", name=f"p2wA{t}")
        mms = []
        for wi in range(n0):
            c = c_lo + wi
            reg = psA[:, wi // 2, 256 * (wi % 2):256 * (wi % 2) + 256]
            mms.append((wi // 2, lambda s, e, reg=reg, c=c: nc.tensor.matmul(
                reg, lhsT=kT_sb[:, 128 * c:128 * c + 128], rhs=qoT_sb[:, t],
                start=s, stop=e)))
            if c == t:  # diag: fold causal tri
                mms.append((wi // 2, lambda s, e, reg=reg: nc.tensor.matmul(
                    reg, lhsT=triNegT_sb, rhs=repI2_sb, start=s, stop=e)))
        emit_chain(mms)
        nc.scalar.activation(
            wp[:, 0:n0, :].rearrange("p a b -> p (a b)"),
            psA[:, :, :].rearrange("p a b -> p (a b)")[:, 0:n0 * 256],
            EXP, scale=SCALE)
        if nw == 5:
            psB = ring.tile([128, 2, 512], F32, tag="ring", name=f"p2wB{t}")
            reg = psB[:, 0, 0:256]
            nc.tensor.matmul(reg, lhsT=kT_sb[:, 128 * t:128 * t + 128],
                             rhs=qoT_sb[:, t], start=True, stop=False)
            nc.tensor.matmul(reg, lhsT=triNegT_sb, rhs=repI2_sb,
                             start=False, stop=True)
            nc.scalar.activation(wp[:, 4, :], reg, EXP, scale=SCALE)
        # window-start partial chunk: keep keys with k_local > q_local
        sw0 = None
        if t >= WCH:
            sw0 = scr.tile([128, 256], F16, tag="sw0")
            nc.gpsimd.tensor_mul(sw0, wp[:, 0, :], triS01_sb)
        pa = accp.tile([128, 512], F32, tag="acc", name=f"p2w{t}")
        for wi in range(nw):
            c = c_lo + wi
            for h in range(HPC):
                src = (sw0[:, 128 * h:128 * h + 128] if (wi == 0 and t >= WCH)
                       else wp[:, wi, 128 * h:128 * h + 128])
                nc.tensor.matmul(pa[:, 129 * h:129 * h + 129], lhsT=src,
                                 rhs=vext_sb[:, c, :],
                                 start=(wi == 0 and h == 0),
                                 stop=(wi == nw - 1 and h == HPC - 1))
        nc.vector.tensor_copy(swao_sb[:, t].rearrange("p h w -> p (h w)"),
                              pa[:, 0:258])
        if DEBUG and t == 5:
            nc.sync.dma_start(out=d["dbg_wp5"], in_=wp)

    # ================= Phase 3: sel attention + combine, all tiles =======
    for t in range(NT):
        c_lo = max(0, t - WCH)
        nw = t - c_lo + 1
        ns = c_lo  # sel-only chunks 0..c_lo-1

        ktn = keepp.tile([NB, HPC, 128], F16, tag="ktn")
        for h in range(HPC):
            nc.vector.tensor_scalar(ktn[:, h, :], keepT_all[:, t, :],
                                    -1.0, MASKNEG, op0=ADD, op1=MUL)

        # exact {0,1} keep mask per (key, query) for the window chunks
        me_ps = ring.tile([128, 2, 512], F32, tag="ring", name=f"p3m{t}")
        mms = []
        for wi in range(nw):
            c = c_lo + wi
            reg = me_ps[:, wi // 4, 128 * (wi % 4):128 * (wi % 4) + 128]
            mms.append((wi // 4, lambda s, e, reg=reg, c=c: nc.tensor.matmul(
                reg, lhsT=e2c_sb[:, c, :], rhs=keepT_all[:, t, :],
                start=s, stop=e)))
        emit_chain(mms)
        meF = mep.tile([128, 5, 128], F16, tag="meF")
        n0 = min(nw, 4)
        nc.vector.tensor_copy(
            meF[:, 0:n0, :].rearrange("p a b -> p (a b)"),
            me_ps[:, 0, 0:n0 * 128])
        if nw == 5:
            nc.vector.tensor_copy(meF[:, 4, :], me_ps[:, 1, 0:128])

        sw = selwp.tile([128, 5, HPC, 128], F16, tag="sw")
        nc.gpsimd.tensor_mul(
            sw[:, 0:nw],
            wps[t][:, 0:nw].rearrange("p w (h q) -> p w h q", h=HPC),
            meF[:, 0:nw].unsqueeze(2).to_broadcast((128, nw, HPC, 128)))
        if DEBUG and t == 5:
            nc.sync.dma_start(out=d["dbg_meF5"], in_=meF)
            nc.sync.dma_start(out=d["dbg_sw5"], in_=sw)
            nc.sync.dma_start(out=d["dbg_ktn5"], in_=ktn)

        sp = None
        if ns > 0:
            sp = sprob.tile([128, 12, 256], F16, tag="sp")
            for cq in range(0, ns, 4):
                n = min(4, ns - cq)
                ps = ring.tile([128, 2, 512], F32, tag="ring",
                               name=f"p3s{t}_{cq}")
                mms = []
                for i in range(n):
                    c = cq + i
                    reg = ps[:, i // 2, 256 * (i % 2):256 * (i % 2) + 256]
                    mms.append((i // 2, lambda s, e, reg=reg, c=c:
                                nc.tensor.matmul(
                        reg, lhsT=kT_sb[:, 128 * c:128 * c + 128],
                        rhs=qoT_sb[:, t], start=s, stop=e)))
                    mms.append((i // 2, lambda s, e, reg=reg, c=c:
                                nc.tensor.matmul(
                        reg, lhsT=e2c_sb[:, c, :], rhs=ktn, start=s, stop=e)))
                emit_chain(mms)
                nc.scalar.activation(
                    sp[:, cq:cq + n, :].rearrange("p a b -> p (a b)"),
                    ps[:, :, :].rearrange("p a b -> p (a b)")[:, 0:n * 256],
                    EXP, scale=SCALE)
            if DEBUG and t == 15:
                nc.sync.dma_start(out=d["dbg_sp15"][:, 0:11], in_=sp[:, 0:11])

        pa = accp.tile([128, 512], F32, tag="acc", name=f"p3a{t}")
        for c in range(t + 1):
            for h in range(HPC):
                if c < c_lo:
                    src = sp[:, c, 128 * h:128 * h + 128]
                else:
                    src = sw[:, c - c_lo, h, :]
                nc.tensor.matmul(pa[:, 129 * h:129 * h + 129], lhsT=src,
                                 rhs=vext_sb[:, c, :],
                                 start=(c == 0 and h == 0),
                                 stop=(c == t and h == HPC - 1))

        if DEBUG and t in (0, 5, 15):
            stg = outp.tile([128, 258], F32, tag="dbgstg", name=f"stg{t}")
            nc.vector.tensor_copy(stg, pa[:, 0:258])
            nc.sync.dma_start(out=d[f"dbg_pa{t}"], in_=stg)

        # ---- combine ----
        ot = outp.tile([128, HPC, 128], F32, tag="ot")
        for h in range(HPC):
            rc0 = smallp.tile([128, 1], F32, tag="rc0")
            nc.vector.reciprocal(rc0, cmpo_sb[:, t, h, 128:129])
            rc1 = smallp.tile([128, 1], F32, tag="rc1")
            nc.vector.reciprocal(rc1, pa[:, 129 * h + 128:129 * h + 129])
            rc2 = smallp.tile([128, 1], F32, tag="rc2")
            nc.vector.reciprocal(rc2, swao_sb[:, t, h, 128:129])
            rs0 = smallp.tile([128, 1], F32, tag="rs0")
            nc.vector.tensor_mul(rs0, rc0, sg_sb[:, t, 3 * h:3 * h + 1])
            rs1 = smallp.tile([128, 1], F32, tag="rs1")
            nc.vector.tensor_mul(rs1, rc1, sg_sb[:, t, 3 * h + 1:3 * h + 2])
            rs2 = smallp.tile([128, 1], F32, tag="rs2")
            nc.vector.tensor_mul(rs2, rc2, sg_sb[:, t, 3 * h + 2:3 * h + 3])
            acc_a = outp.tile([128, 128], F32, tag="acc_a")
            nc.vector.tensor_scalar(acc_a, cmpo_sb[:, t, h, 0:128], rs0,
                                    None, op0=MUL)
            acc_b = outp.tile([128, 128], F32, tag="acc_b")
            nc.vector.scalar_tensor_tensor(
                acc_b, pa[:, 129 * h:129 * h + 128], rs1, acc_a,
                op0=MUL, op1=ADD)
            nc.vector.scalar_tensor_tensor(
                ot[:, h, :], swao_sb[:, t, h, 0:128], rs2, acc_b,
                op0=MUL, op1=ADD)
        nc.sync.dma_start(out=out_ap[t * 128:(t + 1) * 128], in_=ot)
        if DEBUG and t == 5:
            nc.sync.dma_start(out=d["dbg_ot5"], in_=ot)

    if DEBUG:
        nc.sync.dma_start(out=d["dbg_cmpo"], in_=cmpo_sb)
        nc.sync.dma_start(out=d["dbg_swao"], in_=swao_sb)


def _host_inputs(q, k, v, cw):
    """Precompute per-core input arrays (numpy only; no FLOP-heavy work)."""
    q = np.ascontiguousarray(q, np.float32)
    k2 = np.ascontiguousarray(k[:, 0, :], np.float32)
    v2 = np.ascontiguousarray(v[:, 0, :], np.float32)
    cw = np.ascontiguousarray(cw, np.float32)
    f16 = np.float16

    kT = np.ascontiguousarray(k2.T).astype(f16)
    v_ext = np.ascontiguousarray(
        np.concatenate([v2, np.ones((T, 1), np.float32)], 1)
        .reshape(NT, 128, 129).transpose(1, 0, 2)).astype(f16)
    idx = np.arange(M)[:, None] * ST + np.arange(KS)[None, :]
    cmp_k = k2[idx].mean(1)
    cmp_v = v2[idx].mean(1)
    cmp_kT = np.ascontiguousarray(cmp_k.T).astype(f16)
    ju, bu = KS // ST, BS // ST
    j = np.arange(M)[:, None]
    b = np.arange(NB)[None, :]
    ov = np.maximum(0, np.minimum(j + ju, (b + 1) * bu) - np.maximum(j, b * bu))
    Wmap = (ov / ju).astype(np.float32)
    cmp_rhs = np.ascontiguousarray(
        np.concatenate([Wmap, np.ones((M, 1), np.float32), cmp_v,
                        np.ones((M, 1), np.float32)], 1)).astype(f16)

    t_pos = np.arange(T)
    ar = np.arange(128)
    c_idx = np.arange(NT)
    bidx = np.arange(NB)
    e2c = np.ascontiguousarray(
        (bidx[:, None, None] ==
         (2 * c_idx[None, :, None] + ar[None, None, :] // 64)).astype(f16))
    ident = np.eye(128, dtype=np.float32)
    eye16 = np.eye(128, dtype=np.float32).astype(f16)
    repI2 = np.ascontiguousarray(np.tile(eye16, (1, 2)))
    repI16 = np.ascontiguousarray(np.tile(eye16, (1, 16)))
    # triNegT[r, key] = -MASKNEG where key > r (future within diag chunk)
    triNegT = np.ascontiguousarray(
        (-MASKNEG * (ar[None, :] > ar[:, None])).astype(f16))
    # triS01[k, h*128+q] = 1 where k > q (valid at window-start chunk)
    triS01 = np.ascontiguousarray(
        np.tile((ar[:, None] > ar[None, :]).astype(np.float32), (1, 2))
        .astype(f16))
    # cmp validity: keep[m, t] = (m*ST + KS <= t+1); maskT[q, t, m]
    keep_cmp = ((np.arange(M)[:, None] * ST + KS) <= (t_pos[None, :] + 1))
    cmp_maskT_all = np.ascontiguousarray(
        (-MASKNEG * (1.0 - keep_cmp.astype(np.float32)))
        .T.reshape(NT, 128, M).transpose(1, 0, 2)).astype(f16)

    cur_blk = t_pos // BS
    future = bidx[None, :] > cur_blk[:, None]
    forced = (bidx[None, :] < NINIT) | (
        (bidx[None, :] <= cur_blk[:, None])
        & (bidx[None, :] > cur_blk[:, None] - NLOCAL))
    slc_keep = np.ascontiguousarray(
        (~(future | forced)).astype(np.float32)
        .reshape(NT, 128, NB).transpose(1, 0, 2))
    slc_ovr = np.ascontiguousarray(
        np.where(forced, BIGC, np.where(future, NEGC, 0.0)).astype(np.float32)
        .reshape(NT, 128, NB).transpose(1, 0, 2))

    qt_all = q.reshape(NT, 128, HQ, D).transpose(3, 0, 2, 1)  # [D, t, h, q]

    shared = dict(kT=kT, v_ext=v_ext, cmp_kT=cmp_kT, cmp_rhs=cmp_rhs,
                  e2c=e2c, ident=ident, repI2=repI2, repI16=repI16,
                  triNegT=triNegT, triS01=triS01,
                  cmp_maskT_all=cmp_maskT_all)
    in_maps = []
    for c in range(NCORES):
        hs = slice(c * HPC, (c + 1) * HPC)
        ts = slice(2 * c, 2 * c + 2)
        qt16 = np.ascontiguousarray(
            qt_all[:, ts].transpose(0, 1, 2, 3)).astype(f16)  # [D, 2, h, q]
        qoT = np.ascontiguousarray(qt_all[:, :, hs, :]).astype(f16)
        cw_own = np.ascontiguousarray(
            cw[:, hs, :].reshape(NT, 128, 3 * HPC).transpose(1, 0, 2))
        in_maps.append(dict(
            shared, qt16=qt16, qoT=qoT, cw=cw_own,
            cmp_maskT_own=np.ascontiguousarray(cmp_maskT_all[:, ts, :]),
            slc_keep2=np.ascontiguousarray(slc_keep[:, ts, :]),
            slc_ovr2=np.ascontiguousarray(slc_ovr[:, ts, :])))
    return in_maps


_PROGRAM = None


def _get_program():
    global _PROGRAM
    if _PROGRAM is None:
        _PROGRAM = _build_program()
    return _PROGRAM


def kernel(q, k, v, combine_weight, cu_seqlens, _trace=False):
    nc = _get_program()
    in_maps = _host_inputs(np.asarray(q), np.asarray(k), np.asarray(v),
                           np.asarray(combine_weight))
    res = run_bass_kernel_spmd(nc, in_maps, core_ids=list(range(NCORES)),
                               trace=_trace)
    outs = [res.results[c]["out"] for c in range(NCORES)]
    full = np.concatenate(outs, axis=1).astype(np.float32)
    if _trace:
        kernel._last_results = res
    return full
